# revision 29
# baseline (speedup 1.0000x reference)
"""Trainium2 Bass kernel for nn_Attention_86672440033867 (relative-position attention).

Sharding: head-parallel over 8 NeuronCores (1 head per core, all 16 batches).
Each core computes, for its head h:
  qkvT = w_qkv_h^T @ x^T            (op=96 chains -> qT/kT/vT rows)
  S^T  = k_b q_b^T                  (4-way PE row-tiled: 4 key-chunks on
                                     partition bands 0/32/64/96 concurrently)
  P^T  = exp(SCALE*S^T) * exp(B)^T  (one [128,2048] ACT exp per 4-bank PSUM
                                     tile; bias multiply split DVE/GPSIMD)
  O^T  = v_b^T P^T (+ ones col)     (2-way PE col-tiled: query halves at PSUM
                                     partitions 0:33 and 64:97 of one bank)
  out_partial = (O^T / denom)^T @ w_out_h
Host sums the 8 partial projections and adds b_out.

The relative_index gather is resolved on the host: bias = table[relative_index]
is batch-independent, so exp(bias^T) is computed once per head and kept
resident in SBUF (2 MB bf16), amortized over all 16 batches.
"""
import numpy as np
import ml_dtypes
from contextlib import ExitStack, nullcontext

import concourse.bass as bass
import concourse.mybir as mybir
import concourse.tile as tile
from concourse import bacc
from concourse.bass_utils import run_bass_kernel_spmd

BF16 = mybir.dt.bfloat16
F32 = mybir.dt.float32

HEADS = 8
D = 32          # head dim
INP = 384
OUP = 384
SCALE = D ** -0.5
AF = mybir.ActivationFunctionType


def build_kernel(NB=16, N=1024, num_devices=8, loop_k=0):
    """Build the per-core Bass module. NB = total batches, N = tokens/batch."""
    assert NB % 4 == 0 and N % 1024 == 0
    NJC = N // 128          # key chunks (128) per batch
    IH = 512                # query-column tile width
    NIH = N // IH           # query tiles per batch (2)
    JG = 2                  # key chunks per exp/psum group (2 banks)
    NJG = NJC // JG         # jg groups per (b, ih) (4)
    TOK = NB * N

    nc = bacc.Bacc("TRN2", target_bir_lowering=False, num_devices=num_devices)

    xt_d = nc.dram_tensor("xt", [INP, TOK], BF16, kind="ExternalInput")
    wqkv_d = nc.dram_tensor("wqkv", [3, 128, 96], BF16, kind="ExternalInput")
    wout4_d = nc.dram_tensor("wout4", [128, OUP], BF16, kind="ExternalInput")
    expb_d = nc.dram_tensor("expb", [128, NJC, N], BF16, kind="ExternalInput")
    ident_d = nc.dram_tensor("ident", [128, 32], BF16, kind="ExternalInput")
    outp_d = nc.dram_tensor("outp", [TOK, OUP], BF16, kind="ExternalOutput")

    with tile.TileContext(nc) as tc, ExitStack() as ctx:
        const = ctx.enter_context(tc.tile_pool(name="const", bufs=1))
        big = ctx.enter_context(tc.tile_pool(name="big", bufs=1))

        wqkv_sb = const.tile([128, 3, 96], BF16)
        wout_sb = const.tile([128, OUP], BF16)
        ident_sb = const.tile([128, 32], BF16)
        expb_sb = const.tile([128, NJC, N], BF16)
        for kc in range(3):
            nc.sync.dma_start(wqkv_sb[:, kc, :], wqkv_d.ap()[kc])
        nc.sync.dma_start(wout_sb[:], wout4_d.ap())
        nc.sync.dma_start(ident_sb[:], ident_d.ap())
        nc.sync.dma_start(expb_sb[:], expb_d.ap())

        # Resident activation layouts
        QKV = big.tile([96, TOK], BF16)              # rows: qT 0:32, kT 32:64, vT 64:96
        Q4 = big.tile([128, TOK], BF16)              # qT replicated on bands 0/32/64/96
        K4 = big.tile([128, TOK], BF16)              # kT replicated on bands 0/32/64/96
        V_sb = big.tile([128, NB * NJC * 33], BF16)  # v natural [j,d] per (b,jc) + ones col
        den_nat = big.tile([128, NB * 2 * 4], BF16)  # denominators, natural layout
        recip_nat = big.tile([128, NB * 2 * 4], F32)

        nc.gpsimd.memset(V_sb[:], 1.0)  # ones column pre-fill; v blocks overwritten

        xt_pool = ctx.enter_context(tc.tile_pool(name="xt", bufs=4))
        es_pool = ctx.enter_context(tc.tile_pool(name="es", bufs=6))
        pt_pool = ctx.enter_context(tc.tile_pool(name="pt", bufs=20))
        ot_pool = ctx.enter_context(tc.tile_pool(name="ot", bufs=3))
        out_pool = ctx.enter_context(tc.tile_pool(name="outp", bufs=6))

        vv = V_sb[:].rearrange("p (b j e) -> p b j e", j=NJC, e=33)

        loop = tc.For_i(0, loop_k, 1) if loop_k else nullcontext()
        with tc.tile_pool(name="ps_qkv", bufs=1, space="PSUM") as ps_qkv, \
             tc.tile_pool(name="ps_dots", bufs=2, space="PSUM") as ps_dots, \
             tc.tile_pool(name="ps_av", bufs=1, space="PSUM") as ps_av, \
             tc.tile_pool(name="ps_out", bufs=1, space="PSUM") as ps_out, loop:

            mi = 0   # multiply round-robin counter
            oi = 0   # out-scale round-robin counter

            def proj_tile(b, t):
                """One 512-token QKV projection tile + v transposes."""
                i0 = b * N + t * IH
                xt_t = xt_pool.tile([128, 3, IH], BF16, tag="xt")
                nc.sync.dma_start(
                    xt_t[:],
                    xt_d.ap()[:, i0:i0 + IH].rearrange("(c p) q -> p c q", p=128))
                ps = ps_qkv.tile([128, IH], F32, tag="ps_qkv")
                for kc in range(3):
                    nc.tensor.matmul(ps[0:96, :], wqkv_sb[:, kc, :],
                                     xt_t[:, kc, :],
                                     start=(kc == 0), stop=(kc == 2))
                nc.vector.tensor_copy(QKV[:, i0:i0 + IH], ps[0:96, :])
                # v natural: transpose vT [32,128] blocks -> [128,32]
                vt = ps_qkv.tile([128, 4, 32], BF16, tag="ps_vt")
                for r in range(4):
                    nc.tensor.transpose(
                        vt[:, r, :],
                        QKV[64:96, i0 + r * 128:i0 + (r + 1) * 128],
                        ident_sb[64:96, 0:32],
                        tile_position=(64, 0))
                tbase = t * (IH // 128)
                nc.vector.tensor_copy(vv[:, b, tbase:tbase + 4, 0:32],
                                      vt[:, 0:4, :])

            def proj_band(p):
                """Replicate qT/kT of batch pair p onto the 4 partition bands."""
                lo, hi = p * 2 * N, (p + 1) * 2 * N
                for r in range(4):
                    nc.sync.dma_start(Q4[32 * r:32 * r + 32, lo:hi],
                                      QKV[0:32, lo:hi])
                    nc.sync.dma_start(K4[32 * r:32 * r + 32, lo:hi],
                                      QKV[32:64, lo:hi])

            PTS = {}   # (b, ih, jg) -> pt tile, filled by head, consumed by tail

            def head_round(b, ih, jg):
                """One dots round: 4 packed matmuls + exp + bias multiply."""
                nonlocal mi
                i0 = b * N + ih * IH
                ps = ps_dots.tile([128, JG, IH], F32, tag="ps_dots")
                for r in range(JG):
                    jc = jg * JG + r
                    band = jc % 4
                    nc.tensor.matmul(
                        ps[:, r, :],
                        K4[32 * band:32 * band + 32,
                           b * N + jc * 128:b * N + (jc + 1) * 128],
                        Q4[32 * band:32 * band + 32, i0:i0 + IH],
                        start=True, stop=True,
                        tile_position=(32 * band, 0))
                es = es_pool.tile([128, JG, IH], BF16, tag="es")
                nc.scalar.activation(es[:], ps[:], AF.Exp, scale=float(SCALE))
                pt = pt_pool.tile([128, JG, IH], BF16, tag="pt")
                eng = nc.gpsimd if (mi % 2 == 0) else nc.vector
                mi += 1
                eng.tensor_mul(
                    pt[:], es[:],
                    expb_sb[:, jg * JG:(jg + 1) * JG, ih * IH:(ih + 1) * IH])
                PTS[(b, ih, jg)] = pt

            def tail_av(b, av, jlo, jhi):
                """AV accumulation: query halves col-tiled to 0:33 / 64:97."""
                for jc in range(jlo, jhi):
                    p0 = PTS[(b, 0, jc // JG)][:, jc % JG, :]
                    p1 = PTS[(b, 1, jc // JG)][:, jc % JG, :]
                    nc.tensor.matmul(av[0:33, :], vv[:, b, jc, 0:33], p0,
                                     start=(jc == 0), stop=(jc == NJC - 1),
                                     tile_position=(0, 0))
                    nc.tensor.matmul(av[64:97, :], vv[:, b, jc, 0:33], p1,
                                     start=(jc == 0), stop=(jc == NJC - 1),
                                     tile_position=(0, 64))

            def tail_evac(b, av):
                """O^T evac + denominator spread + reciprocal."""
                ot = ot_pool.tile([128, IH], BF16, tag="ot")
                nc.vector.tensor_copy(ot[0:97, :], av[0:97, :])
                # denominator rows -> natural layout via tiny PE transposes
                dent = ps_qkv.tile([128, 8, 2], BF16, tag="ps_vt")
                for half, prow in ((0, 32), (1, 96)):
                    for cc in range(4):
                        nc.tensor.transpose(
                            dent[:, half * 4 + cc, 0:1],
                            ot[prow:prow + 1, cc * 128:(cc + 1) * 128],
                            ident_sb[prow:prow + 1, 0:1],
                            tile_position=(prow, 0))
                nc.vector.tensor_copy(den_nat[:, b * 8:b * 8 + 8],
                                      dent[:, :, 0])
                nc.vector.reciprocal(recip_nat[:, b * 8:b * 8 + 8],
                                     den_nat[:, b * 8:b * 8 + 8])
                return ot

            def tail_proj(b, ot, clo, chi):
                """Out-projection + normalization for token chunks clo:chi."""
                nonlocal oi
                otile = None
                for c in range(clo, chi):
                    half, cc = divmod(c, 4)
                    po = ps_out.tile([128, 512], F32, tag="ps_out")
                    nc.tensor.matmul(
                        po[:, 0:OUP],
                        ot[64 * half:64 * half + 32, cc * 128:(cc + 1) * 128],
                        wout_sb[64 * half:64 * half + 32, :],
                        start=True, stop=True)
                    if c % 2 == 0:
                        otile = out_pool.tile([128, 2, OUP], BF16, tag="outp")
                    rc = recip_nat[:, b * 8 + c:b * 8 + c + 1]
                    if oi % 7 == 6:
                        nc.scalar.activation(otile[:, c % 2, :], po[:, 0:OUP],
                                             AF.Copy, scale=rc)
                    else:
                        nc.vector.tensor_scalar_mul(otile[:, c % 2, :],
                                                    po[:, 0:OUP], rc)
                    oi += 1
                    if c % 2 == 1:
                        nc.sync.dma_start(
                            outp_d.ap()[b * N + (c - 1) * 128:
                                        b * N + (c + 1) * 128, :].rearrange(
                                "(d p) f -> p d f", p=128),
                            otile[:])

            # Software pipeline: head(b) rounds interleaved with tail(b-1)
            # pieces and proj tiles of batch b+2, so the PE always has ready
            # work while ACT drains exp, and ACT is never starved by proj.
            prev = None   # (b, av) of the unit whose tail is in flight

            def section(b):
                nonlocal prev
                pj = b + 2 if b + 2 < NB else None   # batch to project
                pb = prev[0] if prev else None
                pav = prev[1] if prev else None
                pot = [None]

                def nop():
                    pass

                def t_evac():
                    pot[0] = tail_evac(pb, pav)

                tails = [
                    (lambda: proj_tile(pj, 0)) if pj is not None else nop,
                    (lambda: tail_av(pb, pav, 0, 4)) if prev else nop,
                    (lambda: proj_tile(pj, 1)) if pj is not None else nop,
                    (lambda: tail_av(pb, pav, 4, 8)) if prev else nop,
                    (lambda: proj_band(pj // 2))
                    if pj is not None and pj % 2 == 1 else nop,
                    t_evac if prev else nop,
                    (lambda: tail_proj(pb, pot[0], 0, 4)) if prev else nop,
                    (lambda: tail_proj(pb, pot[0], 4, 8)) if prev else nop,
                ]
                # two tails after each of the first four heads; last four heads
                # run tail-free so the next section's exps queue up behind them
                order = [(0, (0, 1)), (1, (2, 3)), (2, (4, 5)), (3, (6, 7)),
                         (4, ()), (5, ()), (6, ()), (7, ())]
                for hi, ts_idx in order:
                    head_round(b, hi // NJG, hi % NJG)
                    for ti in ts_idx:
                        tails[ti]()
                if prev:
                    for ih in range(NIH):
                        for jg in range(NJG):
                            del PTS[(pb, ih, jg)]
                av = ps_av.tile([128, IH], F32, tag="ps_av")
                prev = (b, av)

            for b in range(2):
                proj_tile(b, 0)
                proj_tile(b, 1)
            proj_band(0)
            for b in range(NB):
                section(b)
            # drain the last unit's tail
            pb, pav = prev
            tail_av(pb, pav, 0, 8)
            pot = tail_evac(pb, pav)
            tail_proj(pb, pot, 0, 8)
    nc.compile()
    return nc


def host_prep(x, w_qkv, relative_bias_table, relative_index, w_out, NB, N):
    """Build per-core input maps."""
    bf = ml_dtypes.bfloat16
    TOK = NB * N
    NJC = N // 128
    xt = np.ascontiguousarray(x.reshape(TOK, INP).T).astype(bf)
    ident = np.tile(np.eye(32, dtype=np.float32), (4, 1)).astype(bf)
    bias_full = relative_bias_table[relative_index]  # [N, N, H]
    in_maps = []
    for h in range(HEADS):
        w96 = np.concatenate(
            [w_qkv[:, h * D:(h + 1) * D],
             w_qkv[:, 256 + h * D:256 + (h + 1) * D],
             w_qkv[:, 512 + h * D:512 + (h + 1) * D]], axis=1)  # [384, 96]
        wqkv3 = np.ascontiguousarray(w96.reshape(3, 128, 96)).astype(bf)
        wout4 = np.tile(w_out[h * D:(h + 1) * D, :], (4, 1)).astype(bf)
        expbT = np.exp(bias_full[:, :, h].T)  # [j, i]
        expb = np.ascontiguousarray(
            expbT.reshape(NJC, 128, N).transpose(1, 0, 2)).astype(bf)
        in_maps.append({
            "xt": xt, "wqkv": wqkv3, "wout4": wout4,
            "expb": expb, "ident": ident,
        })
    return in_maps


_NC_CACHE = {}


def kernel(x, w_qkv, relative_bias_table, w_out, b_out, relative_index):
    x = np.asarray(x, dtype=np.float32)
    w_qkv = np.asarray(w_qkv, dtype=np.float32)
    relative_bias_table = np.asarray(relative_bias_table, dtype=np.float32)
    w_out = np.asarray(w_out, dtype=np.float32)
    b_out = np.asarray(b_out, dtype=np.float32)
    relative_index = np.asarray(relative_index)

    NB, N, _ = x.shape
    key = (NB, N)
    if key not in _NC_CACHE:
        _NC_CACHE[key] = build_kernel(NB=NB, N=N, num_devices=HEADS)
    nc = _NC_CACHE[key]

    in_maps = host_prep(x, w_qkv, relative_bias_table, relative_index, w_out, NB, N)
    res = run_bass_kernel_spmd(nc, in_maps, core_ids=list(range(HEADS)))
    out = np.zeros((NB * N, OUP), np.float32)
    for r in res.results:
        out += r["outp"].astype(np.float32)
    out += b_out[None, :]
    return out.reshape(NB, N, OUP)


# revision 33
# speedup vs baseline: 1.0220x; 1.0220x over previous
"""Trainium2 Bass kernel for nn_Attention_86672440033867 (relative-position attention).

Sharding: head-parallel over 8 NeuronCores (1 head per core, all 16 batches).
Each core computes, for its head h:
  qkvT = w_qkv_h^T @ x^T           (M=96 chains -> qT/kT/vT rows)
  S^T  = k_b q_b^T                 (K=32 matmuls)
  P^T  = exp(SCALE*S^T) * exp(B)^T (ACT exp + DVE/GPSIMD multiply; bias via
                                    host-gathered exp(bias) table, batch-invariant)
  O^T  = v_b^T P^T (+ ones col -> softmax denominators)
  out_partial = (O^T / denom)^T @ w_out_h
Host sums the 8 partial projections and adds b_out.

The relative_index gather is resolved on the host: bias = table[relative_index]
is batch-independent, so exp(bias^T) is computed once per head and kept
resident in SBUF (2 MB bf16), amortized over all 16 batches.
"""
import numpy as np
import ml_dtypes
from contextlib import ExitStack, nullcontext

import concourse.bass as bass
import concourse.mybir as mybir
import concourse.tile as tile
from concourse import bacc
from concourse.bass_utils import run_bass_kernel_spmd

BF16 = mybir.dt.bfloat16
F32 = mybir.dt.float32

HEADS = 8
D = 32          # head dim
INP = 384
OUP = 384
SCALE = D ** -0.5
AF = mybir.ActivationFunctionType


def build_kernel(NB=16, N=1024, num_devices=8, loop_k=0):
    """Build the per-core Bass module. NB = total batches, N = tokens/batch."""
    assert NB % 4 == 0 and N % 128 == 0
    NJC = N // 128          # key chunks (128) per batch
    IH = min(512, N)        # query-column tile width
    NIH = N // IH           # query tiles per batch
    NTC = IH // 128         # token chunks (128) per query tile
    JG = min(2, NJC)        # j-chunks per exp/psum group
    NJG = (NJC + JG - 1) // JG
    TOK = NB * N

    nc = bacc.Bacc("TRN2", target_bir_lowering=False, num_devices=num_devices)

    xt_d = nc.dram_tensor("xt", [INP, TOK], BF16, kind="ExternalInput")
    wqkv_d = nc.dram_tensor("wqkv", [3, 128, 96], BF16, kind="ExternalInput")
    wout4_d = nc.dram_tensor("wout4", [128, OUP], BF16, kind="ExternalInput")
    expb_d = nc.dram_tensor("expb", [128, NJC, N], BF16, kind="ExternalInput")
    ident_d = nc.dram_tensor("ident", [128, 32], BF16, kind="ExternalInput")
    outp_d = nc.dram_tensor("outp", [TOK, OUP], BF16, kind="ExternalOutput")

    with tile.TileContext(nc) as tc, ExitStack() as ctx:
        const = ctx.enter_context(tc.tile_pool(name="const", bufs=1))
        big = ctx.enter_context(tc.tile_pool(name="big", bufs=1))

        wqkv_sb = const.tile([128, 3, 96], BF16)
        wout_sb = const.tile([128, OUP], BF16)
        ident_sb = const.tile([128, 32], BF16)
        expb_sb = const.tile([128, NJC, N], BF16)
        for kc in range(3):
            nc.sync.dma_start(wqkv_sb[:, kc, :], wqkv_d.ap()[kc])
        nc.sync.dma_start(wout_sb[:], wout4_d.ap())
        nc.sync.dma_start(ident_sb[:], ident_d.ap())
        nc.sync.dma_start(expb_sb[:], expb_d.ap())

        # Resident activation layouts
        QKV = big.tile([96, TOK], BF16)              # rows: qT 0:32, kT 32:64, vT 64:96
        K0 = big.tile([32, TOK], BF16)               # kT re-homed to partitions 0:32
        V_sb = big.tile([128, NB * NJC * 33], BF16)  # v natural [j,d] per (b,jc) + ones col
        OT = big.tile([33, TOK], BF16)               # attn out^T (+ denom row 32)
        den_nat = big.tile([128, NB * NJC], BF16)    # denominators, natural layout
        recip_nat = big.tile([128, NB * NJC], F32)

        nc.gpsimd.memset(V_sb[:], 1.0)  # ones column pre-fill; v blocks overwritten

        xt_pool = ctx.enter_context(tc.tile_pool(name="xt", bufs=8))
        es_pool = ctx.enter_context(tc.tile_pool(name="es", bufs=3))
        pt_pool = ctx.enter_context(tc.tile_pool(name="pt", bufs=2 * NJG + 2))
        out_pool = ctx.enter_context(tc.tile_pool(name="outp", bufs=6))

        # ---------------- Stage A: qkv projections + v transposes ----------------
        loopA = tc.For_i(0, loop_k, 1) if loop_k else nullcontext()
        with tc.tile_pool(name="ps_qkv", bufs=3, space="PSUM") as ps_qkv, \
             tc.tile_pool(name="ps_vt", bufs=1, space="PSUM") as ps_vt, loopA:
            for tch in range(TOK // IH):
                xt_t = xt_pool.tile([128, 3, IH], BF16, tag="xt")
                nc.sync.dma_start(
                    xt_t[:],
                    xt_d.ap()[:, tch * IH:(tch + 1) * IH].rearrange(
                        "(c p) q -> p c q", p=128))
                ps = ps_qkv.tile([128, IH], F32, tag="ps_qkv")
                for kc in range(3):
                    nc.tensor.matmul(ps[0:96, :], wqkv_sb[:, kc, :],
                                     xt_t[:, kc, :],
                                     start=(kc == 0), stop=(kc == 2))
                nc.vector.tensor_copy(QKV[:, tch * IH:(tch + 1) * IH], ps[0:96, :])
            # re-home kT to partitions 0:32 (DMA crosses partitions)
            nc.sync.dma_start(K0[:], QKV[32:64, :])
            # v transposes: vT [32,128] blocks -> v natural [128,32] per (b,jc)
            TG = min(4, NJC)
            for b in range(NB):
                for jg in range(NJC // TG):
                    vt = ps_vt.tile([128, 4, 1024], BF16, tag="ps_vt")
                    for r in range(TG):
                        jc = jg * TG + r
                        nc.tensor.transpose(
                            vt[:, r, 0:32],
                            QKV[64:96, b * N + jc * 128:b * N + (jc + 1) * 128],
                            ident_sb[64:96, 0:32],
                            tile_position=(64, 0))
                    vv = V_sb[:].rearrange("p (b j e) -> p b j e", j=NJC, e=33)
                    nc.vector.tensor_copy(
                        vv[:, b, jg * TG:jg * TG + TG, 0:32],
                        vt[:, 0:TG, 0:32])

        # ---------------- Stage B: attention + output projection ----------------
        loopB = tc.For_i(0, loop_k, 1) if loop_k else nullcontext()
        with tc.tile_pool(name="ps_dots", bufs=2, space="PSUM") as ps_dots, \
             tc.tile_pool(name="ps_av", bufs=2, space="PSUM") as ps_av, \
             tc.tile_pool(name="ps_out", bufs=2, space="PSUM") as ps_out, loopB:
            vv = V_sb[:].rearrange("p (b j e) -> p b j e", j=NJC, e=33)

            def tail1(b, ih, pts):
                """AV + evac + denominator spread/reciprocal for one unit."""
                i0 = b * N + ih * IH
                av = ps_av.tile([128, IH], F32, tag="ps_av")
                for jc in range(NJC):
                    nc.tensor.matmul(
                        av[0:33, :], vv[:, b, jc, 0:33],
                        pts[jc // JG][:, (jc % JG) * IH:(jc % JG + 1) * IH],
                        start=(jc == 0), stop=(jc == NJC - 1))
                nc.vector.tensor_copy(OT[:, i0:i0 + IH], av[0:33, :])
                # denominator row -> natural layout via tiny PE transposes
                # (a DMA spread would serialize on the HWDGE queue)
                dent = ps_av.tile([128, IH], F32, tag="ps_av")
                dv = dent[:, 0:4].bitcast(BF16)          # [128, 8] bf16
                for tcl in range(NTC):
                    nc.tensor.transpose(
                        dv[:, 2 * tcl:2 * tcl + 1],
                        OT[32:33, i0 + tcl * 128:i0 + (tcl + 1) * 128],
                        ident_sb[32:33, 0:1],
                        tile_position=(32, 0))
                nc.vector.tensor_copy(
                    den_nat[:, b * NJC + ih * NTC:b * NJC + (ih + 1) * NTC],
                    dv[:, 0:8:2])
                nc.vector.reciprocal(
                    recip_nat[:, b * NJC + ih * NTC:b * NJC + (ih + 1) * NTC],
                    den_nat[:, b * NJC + ih * NTC:b * NJC + (ih + 1) * NTC])

            def tail2(b, ih, pts):
                """Output projection + normalize for one unit."""
                i0 = b * N + ih * IH
                ot = None
                for tcl in range(NTC):
                    po = ps_out.tile([128, 512], F32, tag="ps_out")
                    nc.tensor.matmul(
                        po[:, 0:OUP],
                        OT[0:32, i0 + tcl * 128:i0 + (tcl + 1) * 128],
                        wout_sb[0:32, :],
                        start=True, stop=True)
                    if tcl % 2 == 0:
                        ot = out_pool.tile([128, 2, OUP], BF16, tag="outp")
                    rc = recip_nat[:, b * NJC + ih * NTC + tcl:
                                   b * NJC + ih * NTC + tcl + 1]
                    if (ih * NTC + tcl) % 5 == 4:
                        nc.scalar.activation(ot[:, tcl % 2, :], po[:, 0:OUP],
                                             AF.Copy, scale=rc)
                    else:
                        nc.vector.tensor_scalar_mul(ot[:, tcl % 2, :],
                                                    po[:, 0:OUP], rc)
                    if tcl % 2 == 1:
                        nc.sync.dma_start(
                            outp_d.ap()[i0 + (tcl - 1) * 128:
                                        i0 + (tcl + 1) * 128, :].rearrange(
                                "(d p) f -> p d f", p=128),
                            ot[:])

            q = []  # 2-deep software pipeline: tail1 at U-1, tail2 at U-2
            mi = 0
            for b in range(NB):
                for ih in range(NIH):
                    i0 = b * N + ih * IH      # token offset of this query tile
                    pts = []
                    for jg in range(NJG):
                        ps = ps_dots.tile([128, JG * IH], F32, tag="ps_dots")
                        for r in range(JG):
                            jc = jg * JG + r
                            nc.tensor.matmul(
                                ps[:, r * IH:(r + 1) * IH],
                                K0[:, b * N + jc * 128:b * N + (jc + 1) * 128],
                                QKV[0:32, i0:i0 + IH],
                                start=True, stop=True)
                        es = es_pool.tile([128, JG * IH], BF16, tag="es")
                        nc.scalar.activation(es[:], ps[:], AF.Exp, scale=float(SCALE))
                        pt = pt_pool.tile([128, JG * IH], BF16, tag="pt")
                        for r in range(JG):
                            jc = jg * JG + r
                            eng = nc.vector if (mi % 2 == 0) else nc.gpsimd
                            mi += 1
                            eng.tensor_mul(
                                pt[:, r * IH:(r + 1) * IH],
                                es[:, r * IH:(r + 1) * IH],
                                expb_sb[:, jc, ih * IH:(ih + 1) * IH])
                        pts.append(pt)
                    q.append((b, ih, pts))
                    if len(q) >= 2:
                        tail1(*q[-2])
                        tail2(*q[-2])
            tail1(*q[-1])
            tail2(*q[-1])
    nc.compile()
    return nc


def host_prep(x, w_qkv, relative_bias_table, relative_index, w_out, NB, N):
    """Build per-core input maps."""
    bf = ml_dtypes.bfloat16
    TOK = NB * N
    NJC = N // 128
    xt = np.ascontiguousarray(x.reshape(TOK, INP).T).astype(bf)
    ident = np.tile(np.eye(32, dtype=np.float32), (4, 1)).astype(bf)
    bias_full = relative_bias_table[relative_index]  # [N, N, H]
    in_maps = []
    for h in range(HEADS):
        w96 = np.concatenate(
            [w_qkv[:, h * D:(h + 1) * D],
             w_qkv[:, 256 + h * D:256 + (h + 1) * D],
             w_qkv[:, 512 + h * D:512 + (h + 1) * D]], axis=1)  # [384, 96]
        wqkv3 = np.ascontiguousarray(w96.reshape(3, 128, 96)).astype(bf)
        wout4 = np.tile(w_out[h * D:(h + 1) * D, :], (4, 1)).astype(bf)
        expbT = np.exp(bias_full[:, :, h].T)  # [j, i]
        expb = np.ascontiguousarray(
            expbT.reshape(NJC, 128, N).transpose(1, 0, 2)).astype(bf)
        in_maps.append({
            "xt": xt, "wqkv": wqkv3, "wout4": wout4,
            "expb": expb, "ident": ident,
        })
    return in_maps


_NC_CACHE = {}


def kernel(x, w_qkv, relative_bias_table, w_out, b_out, relative_index):
    x = np.asarray(x, dtype=np.float32)
    w_qkv = np.asarray(w_qkv, dtype=np.float32)
    relative_bias_table = np.asarray(relative_bias_table, dtype=np.float32)
    w_out = np.asarray(w_out, dtype=np.float32)
    b_out = np.asarray(b_out, dtype=np.float32)
    relative_index = np.asarray(relative_index)

    NB, N, _ = x.shape
    key = (NB, N)
    if key not in _NC_CACHE:
        _NC_CACHE[key] = build_kernel(NB=NB, N=N, num_devices=HEADS)
    nc = _NC_CACHE[key]

    in_maps = host_prep(x, w_qkv, relative_bias_table, relative_index, w_out, NB, N)
    res = run_bass_kernel_spmd(nc, in_maps, core_ids=list(range(HEADS)))
    out = np.zeros((NB * N, OUP), np.float32)
    for r in res.results:
        out += r["outp"].astype(np.float32)
    out += b_out[None, :]
    return out.reshape(NB, N, OUP)


# revision 37
# speedup vs baseline: 1.2309x; 1.2044x over previous
"""Trainium2 Bass kernel for nn_Attention_86672440033867 (relative-position attention).

Sharding: head-parallel over 8 NeuronCores (1 head per core, all 16 batches).
Each core computes, for its head h:
  qkvT = w_qkv_h^T @ x^T           (M=96 chains -> qT/kT/vT rows)
  S^T  = k_b q_b^T                 (K=32 matmuls)
  P^T  = exp(SCALE*S^T) * exp(B)^T (ACT exp + DVE/GPSIMD multiply; bias via
                                    host-gathered exp(bias) table, batch-invariant)
  O^T  = v_b^T P^T (+ ones col -> softmax denominators)
  out_partial = (O^T / denom)^T @ w_out_h
Host sums the 8 partial projections and adds b_out.

The relative_index gather is resolved on the host: bias = table[relative_index]
is batch-independent, so exp(bias^T) is computed once per head and kept
resident in SBUF (2 MB bf16), amortized over all 16 batches.
"""
import numpy as np
import ml_dtypes
from contextlib import ExitStack, nullcontext

import concourse.bass as bass
import concourse.mybir as mybir
import concourse.tile as tile
from concourse import bacc
from concourse.bass_utils import run_bass_kernel_spmd

BF16 = mybir.dt.bfloat16
F32 = mybir.dt.float32

HEADS = 8
D = 32          # head dim
INP = 384
OUP = 384
SCALE = D ** -0.5
AF = mybir.ActivationFunctionType


def build_kernel(NB=16, N=1024, num_devices=8, loop_k=0):
    """Build the per-core Bass module. NB = total batches, N = tokens/batch."""
    assert NB % 4 == 0 and N % 128 == 0
    NJC = N // 128          # key chunks (128) per batch
    IH = min(512, N)        # query-column tile width
    NIH = N // IH           # query tiles per batch
    NTC = IH // 128         # token chunks (128) per query tile
    JG = min(2, NJC)        # j-chunks per exp/psum group
    NJG = (NJC + JG - 1) // JG
    TOK = NB * N

    nc = bacc.Bacc("TRN2", target_bir_lowering=False, num_devices=num_devices)

    xt_d = nc.dram_tensor("xt", [INP, TOK], BF16, kind="ExternalInput")
    wqkv_d = nc.dram_tensor("wqkv", [3, 128, 96], BF16, kind="ExternalInput")
    wout4_d = nc.dram_tensor("wout4", [128, OUP], BF16, kind="ExternalInput")
    expb_d = nc.dram_tensor("expb", [128, NJC, N], BF16, kind="ExternalInput")
    ident_d = nc.dram_tensor("ident", [128, 32], BF16, kind="ExternalInput")
    outp_d = nc.dram_tensor("outp", [TOK, OUP], BF16, kind="ExternalOutput")

    with tile.TileContext(nc) as tc, ExitStack() as ctx:
        const = ctx.enter_context(tc.tile_pool(name="const", bufs=1))
        big = ctx.enter_context(tc.tile_pool(name="big", bufs=1))

        wqkv_sb = const.tile([128, 3, 96], BF16)
        wout_sb = const.tile([128, OUP], BF16)
        ident_sb = const.tile([128, 32], BF16)
        expb_sb = const.tile([128, NJC, N], BF16)
        for kc in range(3):
            nc.sync.dma_start(wqkv_sb[:, kc, :], wqkv_d.ap()[kc])
        nc.sync.dma_start(wout_sb[:], wout4_d.ap())
        nc.sync.dma_start(ident_sb[:], ident_d.ap())
        nc.sync.dma_start(expb_sb[:], expb_d.ap())

        # Resident activation layouts
        QKV = big.tile([96, TOK], BF16)              # rows: qT 0:32, kT 32:64, vT 64:96
        K0 = big.tile([32, TOK], BF16)               # kT re-homed to partitions 0:32
        V_sb = big.tile([128, NB * NJC * 33], BF16)  # v natural [j,d] per (b,jc) + ones col
        OT = big.tile([33, TOK], BF16)               # attn out^T (+ denom row 32)
        den_nat = big.tile([128, NB * NJC], BF16)    # denominators, natural layout
        recip_nat = big.tile([128, NB * NJC], F32)

        nc.gpsimd.memset(V_sb[:], 1.0)  # ones column pre-fill; v blocks overwritten

        xt_pool = ctx.enter_context(tc.tile_pool(name="xt", bufs=8))
        es_pool = ctx.enter_context(tc.tile_pool(name="es", bufs=3))
        pt_pool = ctx.enter_context(tc.tile_pool(name="pt", bufs=2 * NJG + 2))
        out_pool = ctx.enter_context(tc.tile_pool(name="outp", bufs=6))

        # ---------------- Stage A: qkv projections + v transposes ----------------
        loopA = tc.For_i(0, loop_k, 1) if loop_k else nullcontext()
        with tc.tile_pool(name="ps_qkv", bufs=3, space="PSUM") as ps_qkv, \
             tc.tile_pool(name="ps_vt", bufs=1, space="PSUM") as ps_vt, loopA:
            xt_engs = [nc.sync, nc.scalar, nc.gpsimd, nc.scalar]
            for tch in range(TOK // IH):
                xt_t = xt_pool.tile([128, 3, IH], BF16, tag="xt")
                # spread loads across engine DMA queues for bus parallelism
                xt_engs[tch % 4].dma_start(
                    xt_t[:],
                    xt_d.ap()[:, tch * IH:(tch + 1) * IH].rearrange(
                        "(c p) q -> p c q", p=128))
                ps = ps_qkv.tile([128, IH], F32, tag="ps_qkv")
                for kc in range(3):
                    nc.tensor.matmul(ps[0:96, :], wqkv_sb[:, kc, :],
                                     xt_t[:, kc, :],
                                     start=(kc == 0), stop=(kc == 2))
                nc.vector.tensor_copy(QKV[:, tch * IH:(tch + 1) * IH], ps[0:96, :])
            # re-home kT to partitions 0:32 (DMA crosses partitions)
            nc.sync.dma_start(K0[:], QKV[32:64, :])
            # v transposes: vT [32,128] blocks -> v natural [128,32] per (b,jc)
            TG = min(4, NJC)
            for b in range(NB):
                for jg in range(NJC // TG):
                    vt = ps_vt.tile([128, 4, 1024], BF16, tag="ps_vt")
                    for r in range(TG):
                        jc = jg * TG + r
                        nc.tensor.transpose(
                            vt[:, r, 0:32],
                            QKV[64:96, b * N + jc * 128:b * N + (jc + 1) * 128],
                            ident_sb[64:96, 0:32],
                            tile_position=(64, 0))
                    vv = V_sb[:].rearrange("p (b j e) -> p b j e", j=NJC, e=33)
                    nc.vector.tensor_copy(
                        vv[:, b, jg * TG:jg * TG + TG, 0:32],
                        vt[:, 0:TG, 0:32])

        # ---------------- Stage B: attention + output projection ----------------
        loopB = tc.For_i(0, loop_k, 1) if loop_k else nullcontext()
        with tc.tile_pool(name="ps_dots", bufs=2, space="PSUM") as ps_dots, \
             tc.tile_pool(name="ps_av", bufs=2, space="PSUM") as ps_av, \
             tc.tile_pool(name="ps_out", bufs=2, space="PSUM") as ps_out, loopB:
            vv = V_sb[:].rearrange("p (b j e) -> p b j e", j=NJC, e=33)

            def tail1(b, ih, pts):
                """AV + evac + denominator spread/reciprocal for one unit."""
                i0 = b * N + ih * IH
                av = ps_av.tile([128, IH], F32, tag="ps_av")
                for jc in range(NJC):
                    nc.tensor.matmul(
                        av[0:33, :], vv[:, b, jc, 0:33],
                        pts[jc // JG][:, (jc % JG) * IH:(jc % JG + 1) * IH],
                        start=(jc == 0), stop=(jc == NJC - 1))
                nc.vector.tensor_copy(OT[:, i0:i0 + IH], av[0:33, :])
                # denominator row -> natural layout via tiny PE transposes
                # (a DMA spread would serialize on the HWDGE queue)
                dent = ps_av.tile([128, IH], F32, tag="ps_av")
                dv = dent[:, 0:4].bitcast(BF16)          # [128, 8] bf16
                for tcl in range(NTC):
                    nc.tensor.transpose(
                        dv[:, 2 * tcl:2 * tcl + 1],
                        OT[32:33, i0 + tcl * 128:i0 + (tcl + 1) * 128],
                        ident_sb[32:33, 0:1],
                        tile_position=(32, 0))
                nc.vector.tensor_copy(
                    den_nat[:, b * NJC + ih * NTC:b * NJC + (ih + 1) * NTC],
                    dv[:, 0:8:2])
                nc.vector.reciprocal(
                    recip_nat[:, b * NJC + ih * NTC:b * NJC + (ih + 1) * NTC],
                    den_nat[:, b * NJC + ih * NTC:b * NJC + (ih + 1) * NTC])

            def tail2(b, ih, pts):
                """Output projection + normalize for one unit."""
                i0 = b * N + ih * IH
                ot = None
                for tcl in range(NTC):
                    po = ps_out.tile([128, 512], F32, tag="ps_out")
                    nc.tensor.matmul(
                        po[:, 0:OUP],
                        OT[0:32, i0 + tcl * 128:i0 + (tcl + 1) * 128],
                        wout_sb[0:32, :],
                        start=True, stop=True)
                    if tcl % 2 == 0:
                        ot = out_pool.tile([128, 2, OUP], BF16, tag="outp")
                    rc = recip_nat[:, b * NJC + ih * NTC + tcl:
                                   b * NJC + ih * NTC + tcl + 1]
                    if (ih * NTC + tcl) % 5 == 4:
                        nc.scalar.activation(ot[:, tcl % 2, :], po[:, 0:OUP],
                                             AF.Copy, scale=rc)
                    else:
                        nc.vector.tensor_scalar_mul(ot[:, tcl % 2, :],
                                                    po[:, 0:OUP], rc)
                    if tcl % 2 == 1:
                        oeng = [nc.sync, nc.gpsimd][
                            (b * NJC + ih * NTC + tcl) % 2]
                        oeng.dma_start(
                            outp_d.ap()[i0 + (tcl - 1) * 128:
                                        i0 + (tcl + 1) * 128, :].rearrange(
                                "(d p) f -> p d f", p=128),
                            ot[:])

            q = []  # 2-deep software pipeline: tail1 at U-1, tail2 at U-2
            mi = 0
            for b in range(NB):
                for ih in range(NIH):
                    i0 = b * N + ih * IH      # token offset of this query tile
                    pts = []
                    for jg in range(NJG):
                        ps = ps_dots.tile([128, JG * IH], F32, tag="ps_dots")
                        for r in range(JG):
                            jc = jg * JG + r
                            nc.tensor.matmul(
                                ps[:, r * IH:(r + 1) * IH],
                                K0[:, b * N + jc * 128:b * N + (jc + 1) * 128],
                                QKV[0:32, i0:i0 + IH],
                                start=True, stop=True)
                        es = es_pool.tile([128, JG * IH], BF16, tag="es")
                        nc.scalar.activation(es[:], ps[:], AF.Exp, scale=float(SCALE))
                        pt = pt_pool.tile([128, JG * IH], BF16, tag="pt")
                        for r in range(JG):
                            jc = jg * JG + r
                            eng = nc.vector if (mi % 2 == 0) else nc.gpsimd
                            mi += 1
                            eng.tensor_mul(
                                pt[:, r * IH:(r + 1) * IH],
                                es[:, r * IH:(r + 1) * IH],
                                expb_sb[:, jc, ih * IH:(ih + 1) * IH])
                        pts.append(pt)
                    q.append((b, ih, pts))
                    if len(q) >= 2:
                        tail1(*q[-2])
                        tail2(*q[-2])
            tail1(*q[-1])
            tail2(*q[-1])
    nc.compile()
    return nc


def host_prep(x, w_qkv, relative_bias_table, relative_index, w_out, NB, N):
    """Build per-core input maps."""
    bf = ml_dtypes.bfloat16
    TOK = NB * N
    NJC = N // 128
    xt = np.ascontiguousarray(x.reshape(TOK, INP).T).astype(bf)
    ident = np.tile(np.eye(32, dtype=np.float32), (4, 1)).astype(bf)
    bias_full = relative_bias_table[relative_index]  # [N, N, H]
    in_maps = []
    for h in range(HEADS):
        w96 = np.concatenate(
            [w_qkv[:, h * D:(h + 1) * D],
             w_qkv[:, 256 + h * D:256 + (h + 1) * D],
             w_qkv[:, 512 + h * D:512 + (h + 1) * D]], axis=1)  # [384, 96]
        wqkv3 = np.ascontiguousarray(w96.reshape(3, 128, 96)).astype(bf)
        wout4 = np.tile(w_out[h * D:(h + 1) * D, :], (4, 1)).astype(bf)
        expbT = np.exp(bias_full[:, :, h].T)  # [j, i]
        expb = np.ascontiguousarray(
            expbT.reshape(NJC, 128, N).transpose(1, 0, 2)).astype(bf)
        in_maps.append({
            "xt": xt, "wqkv": wqkv3, "wout4": wout4,
            "expb": expb, "ident": ident,
        })
    return in_maps


_NC_CACHE = {}


def kernel(x, w_qkv, relative_bias_table, w_out, b_out, relative_index):
    x = np.asarray(x, dtype=np.float32)
    w_qkv = np.asarray(w_qkv, dtype=np.float32)
    relative_bias_table = np.asarray(relative_bias_table, dtype=np.float32)
    w_out = np.asarray(w_out, dtype=np.float32)
    b_out = np.asarray(b_out, dtype=np.float32)
    relative_index = np.asarray(relative_index)

    NB, N, _ = x.shape
    key = (NB, N)
    if key not in _NC_CACHE:
        _NC_CACHE[key] = build_kernel(NB=NB, N=N, num_devices=HEADS)
    nc = _NC_CACHE[key]

    in_maps = host_prep(x, w_qkv, relative_bias_table, relative_index, w_out, NB, N)
    res = run_bass_kernel_spmd(nc, in_maps, core_ids=list(range(HEADS)))
    out = np.zeros((NB * N, OUP), np.float32)
    for r in res.results:
        out += r["outp"].astype(np.float32)
    out += b_out[None, :]
    return out.reshape(NB, N, OUP)
